# revision 29
# baseline (speedup 1.0000x reference)
"""CrossAttention kernel for 8 Trainium2 NeuronCores (Bass/Tile).

Problem (hardcoded): x [4,2048,1024] f32, context [4,2048,1024] f32,
mask [4,2048] bool, Wq/Wk/Wv [1024,512], Wo [512,1024], bo [1024].
8 heads x 64 dim, scale 1/8, out = softmax(q k^T * s + maskbias) v @ Wo + bo.

Sharding: core c -> (batch b = c//2, head-group hg = c%2 of 4 heads).
Each core computes a partial output [2048,1024] (its 4 heads through its
256-row slice of Wo); the host sums core pairs and adds bo.

Device-side layout trick: everything is computed in "transposed" form so
no on-device transposes are needed:
  qT/kT = W^T @ x^T come out of the projection matmul as [d, rows].
  sim is computed as simT [j, i]  (lhsT=kT tile, rhs=qT tile), so the
  attention scale fuses into the ACT exp (exp(sim*scale)).
  PV uses expT directly as the moving operand with v' = [v | ones] as the
  stationary one; the ones column yields the softmax denominator for free.
  The PV output [d, i] is exactly the lhsT the Wo projection needs.

Mask handling is free: the host drops masked context rows (softmax weight
exactly zero) and pads to a multiple of 128 with all-zero context rows.
Zero context rows give k=0 -> sim=0 -> exp=1, and the ones column is 0 on
pad rows (host-supplied), so pads contribute nothing to numerator or
denominator. No bias tensor, no ACT bias read.

Out-projection for i-slice ic is emitted one ic-block late: by then its
inputs (the normalized oT rows) are long since ready, so its "sim"-tag
psum slots allocate immediately and its matmuls act as pure filler for
the PE bubbles inside the ACT-bound attention loop (the baseline emitted
it in-line, where the slow normalize chain stalled the next block's sims
behind the out-proj psum ring slots for ~5us + a PE p-state ramp).
"""

import math

import numpy as np
import ml_dtypes

BF16 = ml_dtypes.bfloat16

B, N, DIM = 4, 2048, 1024
HEADS, DH = 8, 64
INNER = HEADS * DH  # 512
HG = INNER // 2  # 256 per head-group

_PROGRAMS: dict[tuple, object] = {}


def _build_program(m_pad: int, repeats: int = 1):
    import concourse.tile as tile
    from concourse import bacc, mybir

    f32 = mybir.dt.float32
    f32r = mybir.dt.float32r
    bf16 = mybir.dt.bfloat16
    Exp = mybir.ActivationFunctionType.Exp
    mpt = m_pad // 128

    nc = bacc.Bacc("TRN2", target_bir_lowering=False, debug=False)
    xT_d = nc.dram_tensor("xT", [DIM, N], bf16, kind="ExternalInput").ap()
    cT_d = nc.dram_tensor("ctxT", [DIM, m_pad], bf16, kind="ExternalInput").ap()
    wq_d = nc.dram_tensor("wq", [DIM, HG], bf16, kind="ExternalInput").ap()
    wk_d = nc.dram_tensor("wk", [DIM, HG], bf16, kind="ExternalInput").ap()
    wv_d = nc.dram_tensor("wv", [DIM, HG], bf16, kind="ExternalInput").ap()
    wo_d = nc.dram_tensor("wo", [HG, DIM], bf16, kind="ExternalInput").ap()
    ones_d = nc.dram_tensor("ones", [128, mpt], bf16, kind="ExternalInput").ap()
    out_d = nc.dram_tensor("out", [N, DIM], f32, kind="ExternalOutput").ap()

    with tile.TileContext(nc) as tc:
        with tc.tile_pool(name="const", bufs=1) as const, tc.tile_pool(
            name="work", bufs=4
        ) as work, tc.tile_pool(name="outp", bufs=3) as outp:
            xT = const.tile([128, 8, N], bf16)
            cT = const.tile([128, 8, m_pad], bf16)
            wq = const.tile([128, 8, HG], bf16)
            wk = const.tile([128, 8, HG], bf16)
            wv = const.tile([128, 8, HG], bf16)
            wo = const.tile([128, 2, DIM], bf16)
            qT = const.tile([128, 2, N], bf16)
            kT = const.tile([128, 2, m_pad], bf16)
            vp = const.tile([128, mpt, 4, DH + 1], bf16)
            oT = const.tile([128, 2, N], bf16)

            # DMA order matters for the pipeline head: weights first (tiny,
            # unblock the projection matmuls), then context (v/k-proj), then
            # x (q-proj is needed later than v/k).
            for kt in range(8):
                s = slice(kt * 128, (kt + 1) * 128)
                nc.sync.dma_start(out=wv[:, kt, :], in_=wv_d[s, :])
                nc.sync.dma_start(out=wk[:, kt, :], in_=wk_d[s, :])
                nc.sync.dma_start(out=wq[:, kt, :], in_=wq_d[s, :])
            for lh in range(4):
                nc.sync.dma_start(out=vp[:, :, lh, DH], in_=ones_d[:, :])
            for kt in range(8):
                s = slice(kt * 128, (kt + 1) * 128)
                nc.sync.dma_start(out=cT[:, kt, :], in_=cT_d[s, :])
            for kt in range(8):
                s = slice(kt * 128, (kt + 1) * 128)
                nc.sync.dma_start(out=xT[:, kt, :], in_=xT_d[s, :])
            nc.sync.dma_start(out=wo[:, 0, :], in_=wo_d[0:128, :])
            nc.sync.dma_start(out=wo[:, 1, :], in_=wo_d[128:256, :])

            def emit_body(psp, emit_v=True, tail_v=False):
                # One persistent psum pool for the whole body so phases
                # overlap on pure dataflow deps (no pool-boundary WAR
                # barriers). Budget: tag "sim" [128,1024] x2 bufs = 4 banks,
                # tags "acc0"/"acc1" x1 buf = 2 banks, tag "proj"
                # [128,512] x2 bufs = 2 banks -> 8 exactly.
                #
                # All non-attention PE work (k/q projections, out-proj) is
                # emitted as "filler" generators pumped 2 matmuls per jt
                # inside the attention loop: the attention steady state is
                # ACT(exp)-gated with ~200-400ns PE bubbles per jt, and the
                # scheduler places filler (priority = emission order) right
                # into those bubbles, keeping the PE p-state hot.
                import collections

                fillers = collections.deque()
                budget = [0]

                def pump(nmm):
                    budget[0] += nmm
                    while fillers and budget[0] > 0:
                        try:
                            budget[0] -= next(fillers[0])
                        except StopIteration:
                            fillers.popleft()

                def drain():
                    budget[0] = 0
                    while fillers:
                        try:
                            next(fillers[0])
                        except StopIteration:
                            fillers.popleft()

                def gen_proj(w, pr, dst, cs, src, jl):
                    # dst[:, pr, cs] = (w[:,:,128pr:] )^T @ src[:, :, cs]
                    ws = slice(pr * 128, (pr + 1) * 128)
                    ps = psp.tile([128, 512], f32, tag="proj", name="ps")
                    for kt in range(8):
                        nc.tensor.matmul(
                            ps[:, :jl],
                            lhsT=w[:, kt, ws],
                            rhs=src[:, kt, cs],
                            start=(kt == 0),
                            stop=(kt == 7),
                        )
                        if kt % 2 == 1:
                            yield 2
                    nc.vector.tensor_copy(out=dst[:, pr, cs], in_=ps[:, :jl])
                    yield 0

                def gen_po(ic):
                    # ---- output projection for i-slice ic -----------------
                    # Pure filler: by pump time its inputs (normalized oT
                    # rows) are ready, and the 8 MB writeback DMA overlaps
                    # attention compute.
                    for it in range(ic * 4, ic * 4 + 4):
                        ts_ = slice(it * 128, (it + 1) * 128)
                        ob = outp.tile([128, DIM], f32, tag="ob", name="ob")
                        for nh2 in range(2):
                            ns = slice(nh2 * 512, (nh2 + 1) * 512)
                            ph = psp.tile(
                                [128, 512], f32, tag="proj", name="ph"
                            )
                            for ck2 in range(2):
                                nc.tensor.matmul(
                                    ph[:, :],
                                    lhsT=oT[:, ck2, ts_],
                                    rhs=wo[:, ck2, ns],
                                    start=(ck2 == 0),
                                    stop=(ck2 == 1),
                                )
                            yield 2
                            nc.vector.tensor_copy(out=ob[:, ns], in_=ph[:, :])
                        nc.sync.dma_start(out=out_d[ts_, :], in_=ob[:, :])
                        yield 0

                # ---- v projection (needed first by attention PV) ----------
                # As a generator so the NEXT body can pipeline it into the
                # tail of the CURRENT body (the per-jt vp overwrite waits on
                # the previous body's last PV reads, which complete
                # progressively through the last attention block).
                def gen_vproj(jt):
                    js = slice(jt * 128, (jt + 1) * 128)
                    ps = psp.tile([128, 4, DH], f32, tag="proj", name="psv")
                    for kt in range(8):
                        nc.tensor.matmul(
                            ps[:, :, :],
                            lhsT=cT[:, kt, js],
                            rhs=wv[:, kt, :],
                            start=(kt == 0),
                            stop=(kt == 7),
                        )
                        if kt % 2 == 1:
                            yield 2
                    nc.vector.tensor_copy(
                        out=vp[:, jt, :, 0:DH], in_=ps[:, :, :]
                    )
                    yield 0

                if emit_v:
                    for jt in range(mpt):
                        for _ in gen_vproj(jt):
                            pass

                # k(0) and q(0, ic=0) eagerly (attn(0,0) needs them); the
                # rest of the projections become interleaved filler.
                kchunks = []
                j0 = 0
                while j0 < m_pad:
                    jl = min(512, m_pad - j0)
                    kchunks.append((slice(j0, j0 + jl), jl))
                    j0 += jl
                for cs, jl in kchunks:
                    for _ in gen_proj(wk, 0, kT, cs, cT, jl):
                        pass
                for _ in gen_proj(wq, 0, qT, slice(0, 512), xT, 512):
                    pass
                for icc in range(1, N // 512):
                    cs = slice(icc * 512, (icc + 1) * 512)
                    fillers.append(gen_proj(wq, 0, qT, cs, xT, 512))
                for cs, jl in kchunks:
                    fillers.append(gen_proj(wk, 1, kT, cs, cT, jl))
                for icc in range(N // 512):
                    cs = slice(icc * 512, (icc + 1) * 512)
                    fillers.append(gen_proj(wq, 1, qT, cs, xT, 512))

                def attn(pr, ic):
                    # ---- attention for head pair (2pr, 2pr+1) -------------
                    # The two K=64 sim matmuls use PE row-groups 0-1 / 2-3
                    # (auto tile_position from lhsT base partition 0/64),
                    # writing the two 512-halves (= 2 banks) of one psum
                    # tile; one ACT exp covers both heads.
                    i0 = ic * 512
                    qs = slice(i0, i0 + 512)
                    acc0 = psp.tile([65, 512], f32, tag="acc0", bufs=1)
                    acc1 = psp.tile([65, 512], f32, tag="acc1", bufs=1)
                    for jt in range(mpt):
                        js = slice(jt * 128, (jt + 1) * 128)
                        sim = psp.tile([128, 1024], f32, tag="sim")
                        nc.tensor.matmul(
                            sim[:, 0:512],
                            lhsT=kT[0:64, pr, js],
                            rhs=qT[0:64, pr, qs],
                            start=True,
                            stop=True,
                        )
                        nc.tensor.matmul(
                            sim[:, 512:1024],
                            lhsT=kT[64:128, pr, js],
                            rhs=qT[64:128, pr, qs],
                            start=True,
                            stop=True,
                        )
                        ex = work.tile([128, 1024], bf16, tag="exp", bufs=6)
                        nc.scalar.activation(
                            out=ex[:, :],
                            in_=sim[:, :],
                            func=Exp,
                            scale=0.125,
                        )
                        nc.tensor.matmul(
                            acc0[:, :],
                            lhsT=vp[:, jt, 2 * pr, :],
                            rhs=ex[:, 0:512],
                            start=(jt == 0),
                            stop=(jt == mpt - 1),
                        )
                        nc.tensor.matmul(
                            acc1[:, :],
                            lhsT=vp[:, jt, 2 * pr + 1, :],
                            rhs=ex[:, 512:1024],
                            start=(jt == 0),
                            stop=(jt == mpt - 1),
                        )
                        pump(4 if jt == 0 else 2)
                    # normalize: oT = acc[0:64] * (1/acc[64]) bcast.
                    # First evacuate the accs PSUM->SBUF on ScalarE (it sits
                    # next to PSUM and has ~25% slack): this frees the acc
                    # banks ~2us earlier, unblocking the next block's PV
                    # chain, and moves the whole normalize (fast-approx
                    # reciprocal -> gpsimd broadcast -> muls) off the
                    # critical path into SBUF-only ops.
                    sc = work.tile([65, 1024], f32, tag="scc")
                    nc.vector.tensor_copy(out=sc[:, 0:512], in_=acc0[:, :])
                    nc.vector.tensor_copy(out=sc[:, 512:1024], in_=acc1[:, :])
                    # The exact DVE reciprocal is iterative (~8 cycles/elem)
                    # and costs free-size x 8 cycles regardless of partition
                    # count, so reshape the [1,1024] den row onto 64
                    # partitions (free-size 16) via a tiny SBUF DMA first:
                    # 8.5us -> ~0.15us of DVE. All off the critical path.
                    d64 = work.tile([64, 16], f32, tag="d64")
                    nc.sync.dma_start(out=d64[:, :], in_=sc[64:65, :])
                    r64 = work.tile([64, 16], f32, tag="r64")
                    nc.vector.reciprocal(out=r64[:, :], in_=d64[:, :])
                    rc = work.tile([1, 1024], f32, tag="recip")
                    nc.sync.dma_start(out=rc[:, :], in_=r64[:, :])
                    bc = work.tile([64, 1024], f32, tag="bcast")
                    nc.gpsimd.partition_broadcast(bc[:, :], rc[:, :])
                    nc.vector.tensor_mul(
                        oT[0:64, pr, qs], sc[0:64, 0:512], bc[:, 0:512]
                    )
                    st = work.tile([64, 512], bf16, tag="stage")
                    nc.vector.tensor_mul(
                        st[:, :], sc[0:64, 512:1024], bc[:, 512:1024]
                    )
                    nc.sync.dma_start(out=oT[64:128, pr, qs], in_=st[:, :])

                # ---- drive attention, pr-major; po(ic) becomes filler -----
                for pr in range(2):
                    for ic in range(N // 512):
                        attn(pr, ic)
                        if pr == 1:
                            fillers.append(gen_po(ic))
                if tail_v:
                    # cross-body pipelining: emit the NEXT body's v
                    # projection into this body's tail so it overlaps the
                    # last attention block instead of running serially at
                    # the next body's head with the ACT engine idle.
                    for jt in range(mpt):
                        fillers.append(gen_vproj(jt))
                drain()

            with tc.tile_pool(name="ps", bufs=2, space="PSUM") as psp:
                for rep in range(repeats):
                    emit_body(psp, emit_v=(rep == 0),
                              tail_v=(rep < repeats - 1))

    nc.compile()
    return nc


def _get_program(m_pad: int, repeats: int = 1):
    key = (m_pad, repeats)
    if key not in _PROGRAMS:
        _PROGRAMS[key] = _build_program(m_pad, repeats)
    return _PROGRAMS[key]


def make_in_maps(x, context, mask, Wq, Wk, Wv, Wo):
    """Host-side sharding: returns (m_pad, list of 8 per-core input dicts)."""
    x = np.asarray(x, dtype=np.float32)
    context = np.asarray(context, dtype=np.float32)
    mask = np.asarray(mask)
    idxs = []
    for b in range(B):
        idx = np.nonzero(mask[b])[0]
        if idx.size == 0:
            # all masked -> reference softmax degenerates to uniform over all
            idx = np.arange(context.shape[1])
        idxs.append(idx)
    m_pad = max(128, 128 * math.ceil(max(i.size for i in idxs) / 128))

    wq8 = np.asarray(Wq, dtype=np.float32)
    wk8 = np.asarray(Wk, dtype=np.float32)
    wv8 = np.asarray(Wv, dtype=np.float32)
    wo8 = np.asarray(Wo, dtype=np.float32)

    in_maps = []
    for c in range(8):
        b, hg = c // 2, c % 2
        idx = idxs[b]
        mb = idx.size
        xT = np.ascontiguousarray(x[b].T).astype(BF16)
        cTt = np.zeros((DIM, m_pad), dtype=BF16)
        cTt[:, :mb] = np.ascontiguousarray(context[b][idx].T)
        onesv = np.zeros((m_pad,), dtype=np.float32)
        onesv[:mb] = 1.0
        ones_t = np.ascontiguousarray(onesv.reshape(m_pad // 128, 128).T)
        s = slice(hg * HG, (hg + 1) * HG)
        in_maps.append(
            {
                "xT": xT,
                "ctxT": cTt,
                "ones": ones_t.astype(BF16),
                "wq": wq8[:, s].astype(BF16),
                "wk": wk8[:, s].astype(BF16),
                "wv": wv8[:, s].astype(BF16),
                "wo": np.ascontiguousarray(wo8[s, :]).astype(BF16),
            }
        )
    return m_pad, in_maps


def kernel(x, context, mask, Wq, Wk, Wv, Wo, bo):
    from concourse.bass_utils import run_bass_kernel_spmd

    m_pad, in_maps = make_in_maps(x, context, mask, Wq, Wk, Wv, Wo)
    nc = _get_program(m_pad)
    res = run_bass_kernel_spmd(nc, in_maps, core_ids=list(range(8))).results
    out = np.empty((B, N, DIM), dtype=np.float32)
    bo32 = np.asarray(bo, dtype=np.float32)
    for b in range(B):
        out[b] = res[2 * b]["out"] + res[2 * b + 1]["out"] + bo32
    return out


# revision 30
# speedup vs baseline: 1.0959x; 1.0959x over previous
"""CrossAttention kernel for 8 Trainium2 NeuronCores (Bass/Tile).

Problem (hardcoded): x [4,2048,1024] f32, context [4,2048,1024] f32,
mask [4,2048] bool, Wq/Wk/Wv [1024,512], Wo [512,1024], bo [1024].
8 heads x 64 dim, scale 1/8, out = softmax(q k^T * s + maskbias) v @ Wo + bo.

Sharding: core c -> (batch b = c//2, head-group hg = c%2 of 4 heads).
Each core computes a partial output [2048,1024] (its 4 heads through its
256-row slice of Wo); the host sums core pairs and adds bo.

Device-side layout trick: everything is computed in "transposed" form so
no on-device transposes are needed:
  qT/kT = W^T @ x^T come out of the projection matmul as [d, rows].
  sim is computed as simT [j, i]  (lhsT=kT tile, rhs=qT tile), so the
  attention scale fuses into the ACT exp (exp(sim*scale)).
  PV uses expT directly as the moving operand with v' = [v | ones] as the
  stationary one; the ones column yields the softmax denominator for free.
  The PV output [d, i] is exactly the lhsT the Wo projection needs.

Mask handling is free: the host drops masked context rows (softmax weight
exactly zero) and pads to a multiple of 128 with all-zero context rows.
Zero context rows give k=0 -> sim=0 -> exp=1, and the ones column is 0 on
pad rows (host-supplied), so pads contribute nothing to numerator or
denominator. No bias tensor, no ACT bias read.

Out-projection for i-slice ic is emitted one ic-block late: by then its
inputs (the normalized oT rows) are long since ready, so its "sim"-tag
psum slots allocate immediately and its matmuls act as pure filler for
the PE bubbles inside the ACT-bound attention loop (the baseline emitted
it in-line, where the slow normalize chain stalled the next block's sims
behind the out-proj psum ring slots for ~5us + a PE p-state ramp).
"""

import math

import numpy as np
import ml_dtypes

BF16 = ml_dtypes.bfloat16

B, N, DIM = 4, 2048, 1024
HEADS, DH = 8, 64
INNER = HEADS * DH  # 512
HG = INNER // 2  # 256 per head-group

_PROGRAMS: dict[tuple, object] = {}


def _build_program(m_pad: int, repeats: int = 1):
    import concourse.tile as tile
    from concourse import bacc, mybir

    f32 = mybir.dt.float32
    f32r = mybir.dt.float32r
    bf16 = mybir.dt.bfloat16
    Exp = mybir.ActivationFunctionType.Exp
    mpt = m_pad // 128

    nc = bacc.Bacc("TRN2", target_bir_lowering=False, debug=False)
    xT_d = nc.dram_tensor("xT", [DIM, N], bf16, kind="ExternalInput").ap()
    cT_d = nc.dram_tensor("ctxT", [DIM, m_pad], bf16, kind="ExternalInput").ap()
    wq_d = nc.dram_tensor("wq", [DIM, HG], bf16, kind="ExternalInput").ap()
    wk_d = nc.dram_tensor("wk", [DIM, HG], bf16, kind="ExternalInput").ap()
    wv_d = nc.dram_tensor("wv", [DIM, HG], bf16, kind="ExternalInput").ap()
    wo_d = nc.dram_tensor("wo", [HG, DIM], bf16, kind="ExternalInput").ap()
    ones_d = nc.dram_tensor("ones", [128, mpt], bf16, kind="ExternalInput").ap()
    out_d = nc.dram_tensor("out", [N, DIM], f32, kind="ExternalOutput").ap()

    with tile.TileContext(nc) as tc:
        with tc.tile_pool(name="const", bufs=1) as const, tc.tile_pool(
            name="work", bufs=4
        ) as work, tc.tile_pool(name="outp", bufs=3) as outp:
            xT = const.tile([128, 8, N], bf16)
            cT = const.tile([128, 8, m_pad], bf16)
            wq = const.tile([128, 8, HG], bf16)
            wk = const.tile([128, 8, HG], bf16)
            wv = const.tile([128, 8, HG], bf16)
            wo = const.tile([128, 2, DIM], bf16)
            qT = const.tile([128, 2, N], bf16)
            kT = const.tile([128, 2, m_pad], bf16)
            vp = const.tile([128, mpt, 4, DH + 1], bf16)
            oT = const.tile([128, 2, N], bf16)

            # DMA order matters for the pipeline head: weights first (tiny,
            # unblock the projection matmuls), then context (v/k-proj), then
            # x (q-proj is needed later than v/k).
            for kt in range(8):
                s = slice(kt * 128, (kt + 1) * 128)
                nc.sync.dma_start(out=wv[:, kt, :], in_=wv_d[s, :])
                nc.sync.dma_start(out=wk[:, kt, :], in_=wk_d[s, :])
                nc.sync.dma_start(out=wq[:, kt, :], in_=wq_d[s, :])
            for lh in range(4):
                nc.sync.dma_start(out=vp[:, :, lh, DH], in_=ones_d[:, :])
            for kt in range(8):
                s = slice(kt * 128, (kt + 1) * 128)
                nc.sync.dma_start(out=cT[:, kt, :], in_=cT_d[s, :])
            for kt in range(8):
                s = slice(kt * 128, (kt + 1) * 128)
                nc.sync.dma_start(out=xT[:, kt, :], in_=xT_d[s, :])
            nc.sync.dma_start(out=wo[:, 0, :], in_=wo_d[0:128, :])
            nc.sync.dma_start(out=wo[:, 1, :], in_=wo_d[128:256, :])

            def emit_body(psp, emit_v=True, tail_v=False):
                # One persistent psum pool for the whole body so phases
                # overlap on pure dataflow deps (no pool-boundary WAR
                # barriers). Budget: tag "sim" [128,1024] x2 bufs = 4 banks,
                # tags "acc0"/"acc1" x1 buf = 2 banks, tag "proj"
                # [128,512] x2 bufs = 2 banks -> 8 exactly.
                #
                # All non-attention PE work (k/q projections, out-proj) is
                # emitted as "filler" generators pumped 2 matmuls per jt
                # inside the attention loop: the attention steady state is
                # ACT(exp)-gated with ~200-400ns PE bubbles per jt, and the
                # scheduler places filler (priority = emission order) right
                # into those bubbles, keeping the PE p-state hot.
                import collections

                fillers = collections.deque()
                budget = [0]

                def pump(nmm):
                    budget[0] += nmm
                    while fillers and budget[0] > 0:
                        try:
                            budget[0] -= next(fillers[0])
                        except StopIteration:
                            fillers.popleft()

                def drain():
                    budget[0] = 0
                    while fillers:
                        try:
                            next(fillers[0])
                        except StopIteration:
                            fillers.popleft()

                def gen_proj(w, pr, dst, cs, src, jl):
                    # dst[:, pr, cs] = (w[:,:,128pr:] )^T @ src[:, :, cs]
                    ws = slice(pr * 128, (pr + 1) * 128)
                    ps = psp.tile([128, 512], f32, tag="proj", name="ps")
                    for kt in range(8):
                        nc.tensor.matmul(
                            ps[:, :jl],
                            lhsT=w[:, kt, ws],
                            rhs=src[:, kt, cs],
                            start=(kt == 0),
                            stop=(kt == 7),
                        )
                        if kt % 2 == 1:
                            yield 2
                    nc.vector.tensor_copy(out=dst[:, pr, cs], in_=ps[:, :jl])
                    yield 0

                def gen_po(ic):
                    # ---- output projection for i-slice ic -----------------
                    # Pure filler: by pump time its inputs (normalized oT
                    # rows) are ready, and the 8 MB writeback DMA overlaps
                    # attention compute.
                    for it in range(ic * 4, ic * 4 + 4):
                        ts_ = slice(it * 128, (it + 1) * 128)
                        ob = outp.tile([128, DIM], f32, tag="ob", name="ob")
                        for nh2 in range(2):
                            ns = slice(nh2 * 512, (nh2 + 1) * 512)
                            ph = psp.tile(
                                [128, 512], f32, tag="proj", name="ph"
                            )
                            for ck2 in range(2):
                                nc.tensor.matmul(
                                    ph[:, :],
                                    lhsT=oT[:, ck2, ts_],
                                    rhs=wo[:, ck2, ns],
                                    start=(ck2 == 0),
                                    stop=(ck2 == 1),
                                )
                            yield 2
                            nc.vector.tensor_copy(out=ob[:, ns], in_=ph[:, :])
                        nc.sync.dma_start(out=out_d[ts_, :], in_=ob[:, :])
                        yield 0

                # ---- v projection (needed first by attention PV) ----------
                # As a generator so the NEXT body can pipeline it into the
                # tail of the CURRENT body (the per-jt vp overwrite waits on
                # the previous body's last PV reads, which complete
                # progressively through the last attention block).
                def gen_vproj(jt):
                    js = slice(jt * 128, (jt + 1) * 128)
                    ps = psp.tile([128, 4, DH], f32, tag="sim", name="psv")
                    for kt in range(8):
                        nc.tensor.matmul(
                            ps[:, :, :],
                            lhsT=cT[:, kt, js],
                            rhs=wv[:, kt, :],
                            start=(kt == 0),
                            stop=(kt == 7),
                        )
                        if kt % 2 == 1:
                            yield 2
                    nc.vector.tensor_copy(
                        out=vp[:, jt, :, 0:DH], in_=ps[:, :, :]
                    )
                    yield 0

                if emit_v:
                    for jt in range(mpt):
                        for _ in gen_vproj(jt):
                            pass

                # k(0) and q(0, ic=0) eagerly (attn(0,0) needs them); the
                # rest of the projections become interleaved filler.
                kchunks = []
                j0 = 0
                while j0 < m_pad:
                    jl = min(512, m_pad - j0)
                    kchunks.append((slice(j0, j0 + jl), jl))
                    j0 += jl
                for cs, jl in kchunks:
                    for _ in gen_proj(wk, 0, kT, cs, cT, jl):
                        pass
                for _ in gen_proj(wq, 0, qT, slice(0, 512), xT, 512):
                    pass
                for icc in range(1, N // 512):
                    cs = slice(icc * 512, (icc + 1) * 512)
                    fillers.append(gen_proj(wq, 0, qT, cs, xT, 512))
                for cs, jl in kchunks:
                    fillers.append(gen_proj(wk, 1, kT, cs, cT, jl))
                for icc in range(N // 512):
                    cs = slice(icc * 512, (icc + 1) * 512)
                    fillers.append(gen_proj(wq, 1, qT, cs, xT, 512))

                def attn(pr, ic):
                    # ---- attention for head pair (2pr, 2pr+1) -------------
                    # The two K=64 sim matmuls use PE row-groups 0-1 / 2-3
                    # (auto tile_position from lhsT base partition 0/64),
                    # writing the two 512-halves (= 2 banks) of one psum
                    # tile; one ACT exp covers both heads.
                    i0 = ic * 512
                    qs = slice(i0, i0 + 512)
                    acc0 = psp.tile([65, 512], f32, tag="acc0", bufs=1)
                    acc1 = psp.tile([65, 512], f32, tag="acc1", bufs=1)
                    for jt in range(mpt):
                        js = slice(jt * 128, (jt + 1) * 128)
                        sim = psp.tile([128, 1024], f32, tag="sim")
                        nc.tensor.matmul(
                            sim[:, 0:512],
                            lhsT=kT[0:64, pr, js],
                            rhs=qT[0:64, pr, qs],
                            start=True,
                            stop=True,
                        )
                        nc.tensor.matmul(
                            sim[:, 512:1024],
                            lhsT=kT[64:128, pr, js],
                            rhs=qT[64:128, pr, qs],
                            start=True,
                            stop=True,
                        )
                        ex = work.tile([128, 1024], bf16, tag="exp", bufs=6)
                        nc.scalar.activation(
                            out=ex[:, :],
                            in_=sim[:, :],
                            func=Exp,
                            scale=0.125,
                        )
                        nc.tensor.matmul(
                            acc0[:, :],
                            lhsT=vp[:, jt, 2 * pr, :],
                            rhs=ex[:, 0:512],
                            start=(jt == 0),
                            stop=(jt == mpt - 1),
                        )
                        nc.tensor.matmul(
                            acc1[:, :],
                            lhsT=vp[:, jt, 2 * pr + 1, :],
                            rhs=ex[:, 512:1024],
                            start=(jt == 0),
                            stop=(jt == mpt - 1),
                        )
                        pump(4 if jt == 0 else 2)
                    # normalize: oT = acc[0:64] * (1/acc[64]) bcast.
                    # First evacuate the accs PSUM->SBUF on ScalarE (it sits
                    # next to PSUM and has ~25% slack): this frees the acc
                    # banks ~2us earlier, unblocking the next block's PV
                    # chain, and moves the whole normalize (fast-approx
                    # reciprocal -> gpsimd broadcast -> muls) off the
                    # critical path into SBUF-only ops.
                    sc = work.tile([65, 1024], f32, tag="scc")
                    nc.vector.tensor_copy(out=sc[:, 0:512], in_=acc0[:, :])
                    nc.vector.tensor_copy(out=sc[:, 512:1024], in_=acc1[:, :])
                    # The exact DVE reciprocal is iterative (~8 cycles/elem)
                    # and costs free-size x 8 cycles regardless of partition
                    # count, so reshape the [1,1024] den row onto 64
                    # partitions (free-size 16) via a tiny SBUF DMA first:
                    # 8.5us -> ~0.15us of DVE. All off the critical path.
                    d64 = work.tile([64, 16], f32, tag="d64")
                    nc.sync.dma_start(out=d64[:, :], in_=sc[64:65, :])
                    r64 = work.tile([64, 16], f32, tag="r64")
                    nc.vector.reciprocal(out=r64[:, :], in_=d64[:, :])
                    rc = work.tile([1, 1024], f32, tag="recip")
                    nc.sync.dma_start(out=rc[:, :], in_=r64[:, :])
                    bc = work.tile([64, 1024], f32, tag="bcast")
                    nc.gpsimd.partition_broadcast(bc[:, :], rc[:, :])
                    nc.vector.tensor_mul(
                        oT[0:64, pr, qs], sc[0:64, 0:512], bc[:, 0:512]
                    )
                    st = work.tile([64, 512], bf16, tag="stage")
                    nc.vector.tensor_mul(
                        st[:, :], sc[0:64, 512:1024], bc[:, 512:1024]
                    )
                    nc.sync.dma_start(out=oT[64:128, pr, qs], in_=st[:, :])

                # ---- drive attention, pr-major; po(ic) becomes filler -----
                for pr in range(2):
                    for ic in range(N // 512):
                        attn(pr, ic)
                        if pr == 1:
                            fillers.append(gen_po(ic))
                drain()

            with tc.tile_pool(name="ps", bufs=2, space="PSUM") as psp:
                for _ in range(repeats):
                    emit_body(psp)

    nc.compile()
    return nc


def _get_program(m_pad: int, repeats: int = 1):
    key = (m_pad, repeats)
    if key not in _PROGRAMS:
        _PROGRAMS[key] = _build_program(m_pad, repeats)
    return _PROGRAMS[key]


def make_in_maps(x, context, mask, Wq, Wk, Wv, Wo):
    """Host-side sharding: returns (m_pad, list of 8 per-core input dicts)."""
    x = np.asarray(x, dtype=np.float32)
    context = np.asarray(context, dtype=np.float32)
    mask = np.asarray(mask)
    idxs = []
    for b in range(B):
        idx = np.nonzero(mask[b])[0]
        if idx.size == 0:
            # all masked -> reference softmax degenerates to uniform over all
            idx = np.arange(context.shape[1])
        idxs.append(idx)
    m_pad = max(128, 128 * math.ceil(max(i.size for i in idxs) / 128))

    wq8 = np.asarray(Wq, dtype=np.float32)
    wk8 = np.asarray(Wk, dtype=np.float32)
    wv8 = np.asarray(Wv, dtype=np.float32)
    wo8 = np.asarray(Wo, dtype=np.float32)

    in_maps = []
    for c in range(8):
        b, hg = c // 2, c % 2
        idx = idxs[b]
        mb = idx.size
        xT = np.ascontiguousarray(x[b].T).astype(BF16)
        cTt = np.zeros((DIM, m_pad), dtype=BF16)
        cTt[:, :mb] = np.ascontiguousarray(context[b][idx].T)
        onesv = np.zeros((m_pad,), dtype=np.float32)
        onesv[:mb] = 1.0
        ones_t = np.ascontiguousarray(onesv.reshape(m_pad // 128, 128).T)
        s = slice(hg * HG, (hg + 1) * HG)
        in_maps.append(
            {
                "xT": xT,
                "ctxT": cTt,
                "ones": ones_t.astype(BF16),
                "wq": wq8[:, s].astype(BF16),
                "wk": wk8[:, s].astype(BF16),
                "wv": wv8[:, s].astype(BF16),
                "wo": np.ascontiguousarray(wo8[s, :]).astype(BF16),
            }
        )
    return m_pad, in_maps


def kernel(x, context, mask, Wq, Wk, Wv, Wo, bo):
    from concourse.bass_utils import run_bass_kernel_spmd

    m_pad, in_maps = make_in_maps(x, context, mask, Wq, Wk, Wv, Wo)
    nc = _get_program(m_pad)
    res = run_bass_kernel_spmd(nc, in_maps, core_ids=list(range(8))).results
    out = np.empty((B, N, DIM), dtype=np.float32)
    bo32 = np.asarray(bo, dtype=np.float32)
    for b in range(B):
        out[b] = res[2 * b]["out"] + res[2 * b + 1]["out"] + bo32
    return out
